# revision 21
# baseline (speedup 1.0000x reference)
# Self-contained Trainium2 Bass kernel for 16-head MultiHeadAttention
# (B=4, L=2048, HIDDEN=1024, 16 heads x d_k=64), sharded 2 heads per core
# across 8 NeuronCores (tensor-parallel on heads; every core sees all tokens).
#
# Per-core plan (all matmuls bf16 with fp32 PSUM accumulation):
#   x and W are bf16-cast AND pre-transposed on the host (x^T, W^T) so the
#   kernel uses only plain contiguous DMAs for them (the DRAM->SBUF XBAR
#   transpose path is ~5x slower than straight DMA and was the hidden wall)
#   Q^T,K^T = W^T-stationary matmuls -> [128 (2 heads x 64), 8192] bf16 (+bias)
#   V^T -> bias -> XBAR transpose [128,128] tiles into scratch (XBAR wiring
#          requires dst column lane == src partition), DVE-copied into
#          V natural [token-part, 2x(64+ones)] (ones col folds the softmax
#          denominator into the AV matmul)
#   S^T tile = K_tile @ Q^T  (head0 on PE rows 0-63, head1 on rows 64-127 run
#          concurrently via tile_position)
#   P^T = exp(S^T/8) on ScalarE straight from PSUM (no max subtraction:
#          |scores| < ~6). ScalarE is the critical engine (~266us of exp), so
#   projection matmuls are emitted one-at-a-time ("pump steps") interleaved
#   inside the attention k-tile loop to fill PE slack under the exp shadow.
#   att^T[65, Lq] += V_aug^T @ P^T  (V stationary; row 64 = denominator)
#   out = att^T[0:64] * broadcast(1/denominator); normalize chain split
#   across DVE (evict, recip) and GpSimd (broadcast, multiply).
#   Output stored transposed (2, 64, 8192); host re-transposes + concats.

import numpy as np

NUM_HEADS = 16
HIDDEN = 1024
D_K = 64
B = 4
L = 2048
N_CORES = 8
HPC = NUM_HEADS // N_CORES      # heads per core = 2
OPC = HPC * D_K                 # output dims per core = 128

P = 128
T = B * L                       # 8192 tokens
KT = HIDDEN // P                # 8 contraction tiles
TCH = 1024                      # token chunk for x transpose/projection
NCH = T // TCH                  # 8 chunks
LKT = L // P                    # 16 key tiles per batch
QC = 512                        # query chunk (one PSUM bank)
LQC = L // QC                   # 4 query chunks per batch

_CACHE = {}


class _ProjUnit:
    """One projection accumulation ([128, 512] narrow or [128, 1024] wide):
    single matmuls emitted as pump steps between attention k-tiles, then a
    bias-add evict (and, for V, transpose-DMAs into the augmented V buffer).
    h2 is None in wide mode (whole chunk per unit)."""

    def __init__(self, env, ch, idx, h2):
        self.env, self.ch, self.idx, self.h2 = env, ch, idx, h2
        self.k = 0
        self.pp = None

    def step(self):
        env = self.env
        nc, dt = env["nc"], env["dt"]
        wide = self.h2 is None
        w = TCH if wide else QC
        if self.pp is None:
            self.pp = env["pjp"].tile([P, w], dt.float32, tag="pj")
        xt = env["xt"][self.ch]
        rhs = xt[:, self.k, :] if wide else (
            xt[:, self.k, self.h2 * QC:(self.h2 + 1) * QC]
        )
        nc.tensor.matmul(
            self.pp[:],
            lhsT=env["wts"][self.idx][:, self.k, :],
            rhs=rhs,
            start=(self.k == 0),
            stop=(self.k == KT - 1),
        )
        self.k += 1
        if self.k == KT:
            if not env["skip_evict"]:
                self._evict()
            return True
        return False

    def _evict(self):
        env = self.env
        nc, dt = env["nc"], env["dt"]
        wide = self.h2 is None
        w = TCH if wide else QC
        t0 = self.ch * TCH + (0 if wide else self.h2 * QC)
        bt = env["bts"][self.idx]
        if self.idx < 2:
            dest = env["qT"] if self.idx == 0 else env["kT"]
            nc.vector.tensor_scalar_add(
                out=dest[:, t0:t0 + w], in0=self.pp[:], scalar1=bt[:]
            )
        else:
            vt = env["vtp"].tile([P, w], dt.bfloat16, tag="vt")
            nc.vector.tensor_scalar_add(out=vt[:], in0=self.pp[:], scalar1=bt[:])
            if env["skip_vtr"]:
                return
            vaug = env["vaug"]
            # XBAR transpose wiring maps src partition p -> dst column lane p,
            # so transpose full-width into a scratch tile (lane offset 0) and
            # slice into vaug's 130-wide layout with cheap bf16 DVE copies.
            for j in range(w // P):
                tt = (t0 + j * P) // P
                vnt = env["vtp"].tile([P, P], dt.bfloat16, tag="vnt")
                nc.sync.dma_start_transpose(vnt[:], vt[:, j * P:(j + 1) * P])
                nc.vector.tensor_copy(out=vaug[:, tt, 0:D_K], in_=vnt[:, 0:D_K])
                nc.vector.tensor_copy(
                    out=vaug[:, tt, D_K + 1:2 * D_K + 1], in_=vnt[:, D_K:2 * D_K]
                )


def _load_xt(env, ch):
    nc, dt = env["nc"], env["dt"]
    xt = env["xtp"].tile([P, KT, TCH], dt.bfloat16, tag="xt")
    # issue big x loads on the Activation HWDGE queue so they don't FIFO-block
    # the small latency-critical DMAs (out writes, V transposes) on SP's queue
    eng = nc.scalar if env["xtq"] else nc.sync
    eng.dma_start(
        out=xt[:],
        in_=env["x16"][ch].rearrange("(k p) t -> p k t", p=P),
    )
    env["xt"][ch] = xt


def _pump(env, n):
    for _ in range(n):
        units = env["units"]
        if not units:
            return
        if units[0].step():
            units.popleft()


def _attn_qchunk(env, b, qc, pump_burst=0):
    nc, dt, AF, ALU = env["nc"], env["dt"], env["AF"], env["ALU"]
    qT, kT, vaug = env["qT"], env["kT"], env["vaug"]
    qs = b * L + qc * QC
    av0 = env["avp"].tile([P, QC], dt.float32, tag="av")
    av1 = env["avp"].tile([P, QC], dt.float32, tag="av")
    pts = {}

    def emit_s(k):
        ks = b * L + k * P
        st = env["scp"].tile([P, 2, QC], dt.float32, tag="sc")
        nc.tensor.matmul(
            st[:, 0, :], lhsT=kT[0:D_K, ks:ks + P], rhs=qT[0:D_K, qs:qs + QC],
            start=True, stop=True, tile_position=(0, 0),
        )
        nc.tensor.matmul(
            st[:, 1, :], lhsT=kT[D_K:P, ks:ks + P], rhs=qT[D_K:P, qs:qs + QC],
            start=True, stop=True, tile_position=(64, 0),
        )
        pt = env["ptp"].tile([P, 2, QC], dt.bfloat16, tag="pt")
        nc.scalar.activation(
            out=pt[:], in_=st[:], func=AF.Exp, scale=1.0 / np.sqrt(D_K)
        )
        pts[k] = pt

    if pump_burst:
        pump_n = 0
    elif env.get("pump_wide"):
        pump_n = 1
    elif env.get("pace4"):
        pump_n = 2 if (qc + b) % 4 != 3 else 1
    elif env.get("wrap"):
        pump_n = 2 if qc < LQC - 1 else 0
    else:
        pump_n = 2 if qc < LQC - 1 else 0
    emit_s(0)
    emit_s(1)
    for k in range(LKT):
        lt = b * LKT + k
        pt = pts.pop(k)
        if not env["skip_av"]:
            nc.tensor.matmul(
                av0[:D_K + 1, :], lhsT=vaug[:, lt, 0:D_K + 1], rhs=pt[:, 0, :],
                start=(k == 0), stop=(k == LKT - 1),
            )
            nc.tensor.matmul(
                av1[:D_K + 1, :], lhsT=vaug[:, lt, D_K + 1:2 * (D_K + 1)],
                rhs=pt[:, 1, :],
                start=(k == 0), stop=(k == LKT - 1),
            )
        if k + 2 < LKT:
            emit_s(k + 2)
        _pump(env, pump_n)

    if env["skip_av"]:
        nc.vector.memset(av0[:, 0:4], 0.0)
        nc.vector.memset(av1[:, 0:4], 0.0)
        return
    for h, av in ((0, av0), (1, av1)):
        # evict PSUM->SBUF immediately so the accumulator bank frees for the
        # next qchunk instead of across the whole normalize chain
        avs = env["fin"].tile([D_K + 1, QC], dt.float32, tag="avs")
        nc.vector.tensor_copy(out=avs[:], in_=av[:D_K + 1, :])
        rc = env["fin"].tile([1, QC], dt.float32, tag="rc")
        nc.vector.reciprocal(rc[:], avs[D_K:D_K + 1, :])
        bc = env["fin"].tile([D_K, QC], dt.float32, tag="bc")
        nc.gpsimd.partition_broadcast(bc[:], rc[:])
        osb = env["fin"].tile([D_K, QC], dt.float32, tag="osb")
        if env["fin_dve"]:
            nc.vector.tensor_tensor(osb[:], avs[0:D_K, :], bc[:], ALU.mult)
        else:
            nc.gpsimd.tensor_tensor(osb[:], avs[0:D_K, :], bc[:], ALU.mult)
        nc.sync.dma_start(out=env["out"][h, :, qs:qs + QC], in_=osb[:])


def _build_nc(reps=1, fin_dve=1, pump_burst=0, skip_av=0, skip_attn=0, skip_proj=0, pump_wide=0, skip_vtr=0, xt_only=0, skip_evict=0, xt_pre=0, av3=0, fin4=0, xtq=1, pace4=0, wrap=0):
    import contextlib
    from collections import deque

    import concourse.bacc as bacc
    import concourse.mybir as mybir
    import concourse.tile as tile

    dt = mybir.dt
    AF = mybir.ActivationFunctionType
    ALU = mybir.AluOpType

    nc = bacc.Bacc(None, target_bir_lowering=False, debug=False)

    x16 = nc.declare_dram_parameter("x16c", [NCH, HIDDEN, TCH], dt.bfloat16, isOutput=False)
    wq = nc.declare_dram_parameter("wqt", [HIDDEN, P], dt.bfloat16, isOutput=False)
    wk = nc.declare_dram_parameter("wkt", [HIDDEN, P], dt.bfloat16, isOutput=False)
    wv = nc.declare_dram_parameter("wvt", [HIDDEN, P], dt.bfloat16, isOutput=False)
    bq = nc.declare_dram_parameter("bq", [P, 1], dt.float32, isOutput=False)
    bk = nc.declare_dram_parameter("bk", [P, 1], dt.float32, isOutput=False)
    bv = nc.declare_dram_parameter("bv", [P, 1], dt.float32, isOutput=False)
    out = nc.declare_dram_parameter("out", [HPC, D_K, T], dt.float32, isOutput=True)

    with tile.TileContext(nc) as tc:
        with (
            tc.tile_pool(name="const", bufs=1) as const,
            tc.tile_pool(name="persist", bufs=1) as persist,
            tc.tile_pool(name="xtp", bufs=8 if xt_pre else 4) as xtp,
            tc.tile_pool(name="vtp", bufs=2) as vtp,
            tc.tile_pool(name="ptp", bufs=3 if xt_pre else 4) as ptp,
            tc.tile_pool(name="fin", bufs=4 if fin4 else 2) as fin,
            # PSUM budget (8 banks): scores 2x2 + proj 2x1 + attended 2x1.
            tc.tile_pool(name="scp", bufs=2, space="PSUM") as scp,
            tc.tile_pool(name="pjp", bufs=1 if (pump_wide or av3) else 2, space="PSUM") as pjp,
            tc.tile_pool(name="avp", bufs=3 if av3 else 2, space="PSUM") as avp,
        ):
            # --- weights: host ships W^T [HIDDEN, 128]; plain strided DMA
            # into [hid-part, kt, 128] (no XBAR transposes needed) ---
            wts, bts = [], []
            for nm, wparam, bparam in (("q", wq, bq), ("k", wk, bk), ("v", wv, bv)):
                wt = const.tile([P, KT, P], dt.bfloat16, tag=f"wt{nm}")
                nc.sync.dma_start(
                    out=wt[:],
                    in_=wparam[:].rearrange("(k p) d -> p k d", p=P),
                )
                bt = const.tile([P, 1], dt.float32, tag=f"b{nm}")
                nc.sync.dma_start(out=bt[:], in_=bparam[:])
                wts.append(wt)
                bts.append(bt)

            qT = persist.tile([P, T], dt.bfloat16, tag="qT")
            kT = persist.tile([P, T], dt.bfloat16, tag="kT")
            vaug = persist.tile([P, T // P, 2 * (D_K + 1)], dt.bfloat16, tag="vaug")
            nc.vector.memset(vaug[:, :, D_K:D_K + 1], 1.0)
            nc.vector.memset(vaug[:, :, 2 * D_K + 1:2 * D_K + 2], 1.0)
            if skip_vtr:
                nc.vector.memset(vaug[:, :, 0:D_K], 0.5)
                nc.vector.memset(vaug[:, :, D_K + 1:2 * D_K + 1], 0.5)
            if wrap:
                nc.vector.memset(qT[:], 0.0)
                nc.vector.memset(kT[:], 0.0)
                nc.vector.memset(vaug[:, :, 0:D_K], 0.0)
                nc.vector.memset(vaug[:, :, D_K + 1:2 * D_K + 1], 0.0)
            if skip_proj or xt_only or skip_evict:
                nc.vector.memset(qT[:], 0.25)
                nc.vector.memset(kT[:], 0.25)
                nc.vector.memset(vaug[:, :, 0:D_K], 0.5)
                nc.vector.memset(vaug[:, :, D_K + 1:2 * D_K + 1], 0.5)

            env = {
                "nc": nc, "dt": dt, "AF": AF, "ALU": ALU,
                "x16": x16, "out": out,
                "wts": wts, "bts": bts,
                "qT": qT, "kT": kT, "vaug": vaug,
                "xtp": xtp, "vtp": vtp, "ptp": ptp, "fin": fin,
                "scp": scp, "pjp": pjp, "avp": avp,
                "xt": {}, "units": deque(),
                "fin_dve": fin_dve, "skip_av": skip_av, "pump_wide": pump_wide,
                "skip_vtr": skip_vtr, "skip_evict": skip_evict, "xtq": xtq,
                "pace4": pace4, "wrap": wrap,
            }

            def queue_units(ch_pair):
                # K first (all keys needed at next batch's first scores),
                # V second (vaug needed by first AV), Q last (consumed
                # progressively per qchunk).
                for idx in (1, 2, 0):
                    for ch in ch_pair:
                        if pump_wide:
                            env["units"].append(_ProjUnit(env, ch, idx, None))
                        else:
                            for h2 in range(TCH // QC):
                                env["units"].append(_ProjUnit(env, ch, idx, h2))

            rep_ctx = tc.For_i(0, reps, 1) if reps > 1 else contextlib.nullcontext()
            with rep_ctx:
                # prologue: first two chunks projected back-to-back (ScalarE
                # has nothing to do yet), chunks 2-3 DMA'd in the background.
                if not skip_proj:
                    if wrap:
                        # steady-state wrap: only ch2/ch3 load here (b0 pumps
                        # them); ch0/ch1 were projected by the previous rep's
                        # b3 and reload via b2's loads each rep.
                        _load_xt(env, 2)
                        _load_xt(env, 3)
                    elif xt_pre:
                        for ch in range(NCH):
                            _load_xt(env, ch)
                    else:
                        for ch in range(4):
                            _load_xt(env, ch)
                    if not xt_only and not wrap:
                        queue_units((0, 1))
                        _pump(env, 12 * KT)

                for b in range(B):
                    # wrap mode (timing reps only): batch 3 pumps the NEXT
                    # repetition's chunk 0/1 projections, so steady-state has
                    # no projection-only prologue and no pump-free batch.
                    nxt = (2 * b + 2) % NCH if wrap else 2 * b + 2
                    if not skip_proj and not xt_only and (wrap or nxt < NCH):
                        queue_units((nxt, nxt + 1))
                    if pump_burst or skip_attn:
                        _pump(env, 12 * KT)
                    for qc in range(LQC):
                        if not skip_proj and not xt_pre:
                            if wrap:
                                if b < B - 1 and qc == 1:
                                    _load_xt(env, (nxt + 2) % NCH)
                                if b < B - 1 and qc == 2:
                                    _load_xt(env, (nxt + 3) % NCH)
                            else:
                                if qc == 1 and nxt + 2 < NCH:
                                    _load_xt(env, nxt + 2)
                                if qc == 2 and nxt + 3 < NCH:
                                    _load_xt(env, nxt + 3)
                        if not skip_attn:
                            _attn_qchunk(env, b, qc, pump_burst or skip_proj)

    nc.compile()
    return nc


def get_nc(reps=1, **kw):
    key = f"nc{reps}-{sorted(kw.items())}"
    if key not in _CACHE:
        _CACHE[key] = _build_nc(reps, **kw)
    return _CACHE[key]


def _shard_inputs(x, Wq, bq, Wk, bk, Wv, bv):
    import ml_dtypes

    x16c = np.ascontiguousarray(
        np.asarray(x, dtype=np.float32)
        .reshape(NCH, TCH, HIDDEN)
        .astype(ml_dtypes.bfloat16)
        .transpose(0, 2, 1)
    )
    in_maps = []
    for c in range(N_CORES):
        sl = slice(c * OPC, (c + 1) * OPC)
        in_maps.append({
            "x16c": x16c,
            "wqt": np.ascontiguousarray(np.asarray(Wq, dtype=np.float32)[sl].astype(ml_dtypes.bfloat16).T),
            "wkt": np.ascontiguousarray(np.asarray(Wk, dtype=np.float32)[sl].astype(ml_dtypes.bfloat16).T),
            "wvt": np.ascontiguousarray(np.asarray(Wv, dtype=np.float32)[sl].astype(ml_dtypes.bfloat16).T),
            "bq": np.ascontiguousarray(np.asarray(bq, dtype=np.float32)[sl].reshape(P, 1)),
            "bk": np.ascontiguousarray(np.asarray(bk, dtype=np.float32)[sl].reshape(P, 1)),
            "bv": np.ascontiguousarray(np.asarray(bv, dtype=np.float32)[sl].reshape(P, 1)),
        })
    return in_maps


def _gather(results):
    att = np.empty((B, NUM_HEADS, L, D_K), dtype=np.float32)
    for c in range(N_CORES):
        r = results[c]["out"]  # (HPC, D_K, T)
        for h in range(HPC):
            att[:, c * HPC + h] = r[h].T.reshape(B, L, D_K)
    return att


def run(x, Wq, bq, Wk, bk, Wv, bv, trace=False):
    from concourse.bass_utils import run_bass_kernel_spmd

    nc = get_nc()
    in_maps = _shard_inputs(x, Wq, bq, Wk, bk, Wv, bv)
    res = run_bass_kernel_spmd(
        nc, in_maps, core_ids=list(range(N_CORES)), trace=trace
    )
    return _gather(res.results), res


def kernel(x, Wq, bq, Wk, bk, Wv, bv):
    att, _ = run(x, Wq, bq, Wk, bk, Wv, bv, trace=False)
    return att


# revision 22
# speedup vs baseline: 1.4255x; 1.4255x over previous
# Self-contained Trainium2 Bass kernel for 16-head MultiHeadAttention
# (B=4, L=2048, HIDDEN=1024, 16 heads x d_k=64), sharded 2 heads per core
# across 8 NeuronCores (tensor-parallel on heads; every core sees all tokens).
#
# Per-core plan (all matmuls bf16 with fp32 PSUM accumulation):
#   x and W are bf16-cast AND pre-transposed on the host (x^T, W^T) so the
#   kernel uses only plain contiguous DMAs for them (the DRAM->SBUF XBAR
#   transpose path is ~5x slower than straight DMA and was the hidden wall)
#   Q^T,K^T = W^T-stationary matmuls -> [128 (2 heads x 64), 8192] bf16 (+bias)
#   V^T -> bias -> XBAR transpose [128,128] tiles into scratch (XBAR wiring
#          requires dst column lane == src partition), DVE-copied into
#          V natural [token-part, 2x(64+ones)] (ones col folds the softmax
#          denominator into the AV matmul)
#   S^T tile = K_tile @ Q^T  (head0 on PE rows 0-63, head1 on rows 64-127 run
#          concurrently via tile_position)
#   P^T = exp(S^T/8) on ScalarE straight from PSUM (no max subtraction:
#          |scores| < ~6). ScalarE is the critical engine (~266us of exp), so
#   projection matmuls are emitted one-at-a-time ("pump steps") interleaved
#   inside the attention k-tile loop to fill PE slack under the exp shadow.
#   att^T[65, Lq] += V_aug^T @ P^T  (V stationary; row 64 = denominator)
#   out = att^T[0:64] * broadcast(1/denominator); normalize chain split
#   across DVE (evict, recip) and GpSimd (broadcast, multiply).
#   Output stored transposed (2, 64, 8192); host re-transposes + concats.

import numpy as np

NUM_HEADS = 16
HIDDEN = 1024
D_K = 64
B = 4
L = 2048
N_CORES = 8
HPC = NUM_HEADS // N_CORES      # heads per core = 2
OPC = HPC * D_K                 # output dims per core = 128

P = 128
T = B * L                       # 8192 tokens
KT = HIDDEN // P                # 8 contraction tiles
TCH = 1024                      # token chunk for x transpose/projection
NCH = T // TCH                  # 8 chunks
LKT = L // P                    # 16 key tiles per batch
QC = 512                        # query chunk (one PSUM bank)
LQC = L // QC                   # 4 query chunks per batch

_CACHE = {}


class _ProjUnit:
    """One projection accumulation ([128, 512] narrow or [128, 1024] wide):
    single matmuls emitted as pump steps between attention k-tiles, then a
    bias-add evict (and, for V, transpose-DMAs into the augmented V buffer).
    h2 is None in wide mode (whole chunk per unit)."""

    def __init__(self, env, ch, idx, h2):
        self.env, self.ch, self.idx, self.h2 = env, ch, idx, h2
        self.k = 0
        self.pp = None

    def step(self):
        env = self.env
        nc, dt = env["nc"], env["dt"]
        wide = self.h2 is None
        w = TCH if wide else QC
        if self.pp is None:
            self.pp = env["pjp"].tile([P, w], dt.float32, tag="pj")
        xt = env["xt"][self.ch]
        rhs = xt[:, self.k, :] if wide else (
            xt[:, self.k, self.h2 * QC:(self.h2 + 1) * QC]
        )
        nc.tensor.matmul(
            self.pp[:],
            lhsT=env["wts"][self.idx][:, self.k, :],
            rhs=rhs,
            start=(self.k == 0),
            stop=(self.k == KT - 1),
        )
        self.k += 1
        if self.k == KT:
            if not env["skip_evict"]:
                self._evict()
            return True
        return False

    def _evict(self):
        env = self.env
        nc, dt = env["nc"], env["dt"]
        wide = self.h2 is None
        w = TCH if wide else QC
        t0 = self.ch * TCH + (0 if wide else self.h2 * QC)
        bt = env["bts"][self.idx]
        if self.idx < 2:
            dest = env["qT"] if self.idx == 0 else env["kT"]
            nc.vector.tensor_scalar_add(
                out=dest[:, t0:t0 + w], in0=self.pp[:], scalar1=bt[:]
            )
        else:
            vt = env["vtp"].tile([P, w], dt.bfloat16, tag="vt")
            nc.vector.tensor_scalar_add(out=vt[:], in0=self.pp[:], scalar1=bt[:])
            if env["skip_vtr"]:
                return
            vaug = env["vaug"]
            # XBAR transpose wiring maps src partition p -> dst column lane p,
            # so transpose full-width into a scratch tile (lane offset 0) and
            # slice into vaug's 130-wide layout with cheap bf16 DVE copies.
            for j in range(w // P):
                tt = (t0 + j * P) // P
                vnt = env["vtp"].tile([P, P], dt.bfloat16, tag="vnt")
                nc.sync.dma_start_transpose(vnt[:], vt[:, j * P:(j + 1) * P])
                nc.vector.tensor_copy(out=vaug[:, tt, 0:D_K], in_=vnt[:, 0:D_K])
                nc.vector.tensor_copy(
                    out=vaug[:, tt, D_K + 1:2 * D_K + 1], in_=vnt[:, D_K:2 * D_K]
                )


def _load_xt(env, ch):
    nc, dt = env["nc"], env["dt"]
    xt = env["xtp"].tile([P, KT, TCH], dt.bfloat16, tag="xt")
    # issue big x loads on the Activation HWDGE queue so they don't FIFO-block
    # the small latency-critical DMAs (out writes, V transposes) on SP's queue
    eng = nc.scalar if env["xtq"] else nc.sync
    eng.dma_start(
        out=xt[:],
        in_=env["x16"][ch].rearrange("(k p) t -> p k t", p=P),
    )
    env["xt"][ch] = xt


def _pump(env, n):
    done = 0
    for _ in range(n):
        units = env["units"]
        if not units:
            return done
        if units[0].step():
            units.popleft()
        done += 1
    return done


def _attn_qchunk(env, b, qc, pump_burst=0):
    nc, dt, AF, ALU = env["nc"], env["dt"], env["AF"], env["ALU"]
    qT, kT, vaug = env["qT"], env["kT"], env["vaug"]
    qs = b * L + qc * QC
    av0 = env["avp"].tile([P, QC], dt.float32, tag="av")
    av1 = env["avp"].tile([P, QC], dt.float32, tag="av")
    pts = {}
    ks0 = b * L

    def emit_s(k):
        ks = b * L + k * P
        st = env["scp"].tile([P, 2, QC], dt.float32, tag="sc")
        nc.tensor.matmul(
            st[:, 0, :], lhsT=kT[0:D_K, ks:ks + P], rhs=qT[0:D_K, qs:qs + QC],
            start=True, stop=True, tile_position=(0, 0),
        )
        nc.tensor.matmul(
            st[:, 1, :], lhsT=kT[D_K:P, ks:ks + P], rhs=qT[D_K:P, qs:qs + QC],
            start=True, stop=True, tile_position=(64, 0),
        )
        pt = env["ptp"].tile([P, 2, QC], dt.bfloat16, tag="pt")
        nc.scalar.activation(
            out=pt[:], in_=st[:], func=AF.Exp, scale=1.0 / np.sqrt(D_K)
        )
        pts[k] = pt

    if pump_burst:
        pump_n = 0
    elif env.get("pump_wide"):
        pump_n = 1
    elif env.get("pace4"):
        pump_n = 2 if (qc + b) % 4 != 3 else 1
    elif env.get("wrap"):
        pump_n = 2 if qc < LQC - 1 else 0
    else:
        pump_n = 2 if qc < LQC - 1 else 0
    emit_s(0)
    emit_s(1)
    for k in range(LKT):
        lt = b * LKT + k
        pt = pts.pop(k)
        if not env["skip_av"]:
            nc.tensor.matmul(
                av0[:D_K + 1, :], lhsT=vaug[:, lt, 0:D_K + 1], rhs=pt[:, 0, :],
                start=(k == 0), stop=(k == LKT - 1),
            )
            nc.tensor.matmul(
                av1[:D_K + 1, :], lhsT=vaug[:, lt, D_K + 1:2 * (D_K + 1)],
                rhs=pt[:, 1, :],
                start=(k == 0), stop=(k == LKT - 1),
            )
        if k + 2 < LKT:
            emit_s(k + 2)
        consumed = _pump(env, max(pump_n, 2 if env["filler"] else 0))
        # keep the PE continuously busy in pump-free windows so it never
        # drops out of its top p-state: dead matmuls into an unread pj tile
        if env["filler"]:
            for _ in range(2 - consumed):
                fl = env["pjp"].tile([P, QC], dt.float32, tag="pj")
                nc.tensor.matmul(
                    fl[:], lhsT=kT[0:D_K, ks0:ks0 + P],
                    rhs=qT[0:D_K, qs:qs + QC], start=True, stop=True,
                )

    if env["skip_av"]:
        nc.vector.memset(av0[:, 0:4], 0.0)
        nc.vector.memset(av1[:, 0:4], 0.0)
        return
    for h, av in ((0, av0), (1, av1)):
        # evict PSUM->SBUF immediately so the accumulator bank frees for the
        # next qchunk instead of across the whole normalize chain
        avs = env["fin"].tile([D_K + 1, QC], dt.float32, tag="avs")
        nc.vector.tensor_copy(out=avs[:], in_=av[:D_K + 1, :])
        rc = env["fin"].tile([1, QC], dt.float32, tag="rc")
        nc.vector.reciprocal(rc[:], avs[D_K:D_K + 1, :])
        bc = env["fin"].tile([D_K, QC], dt.float32, tag="bc")
        nc.gpsimd.partition_broadcast(bc[:], rc[:])
        osb = env["fin"].tile([D_K, QC], dt.float32, tag="osb")
        if env["fin_dve"]:
            nc.vector.tensor_tensor(osb[:], avs[0:D_K, :], bc[:], ALU.mult)
        else:
            nc.gpsimd.tensor_tensor(osb[:], avs[0:D_K, :], bc[:], ALU.mult)
        nc.sync.dma_start(out=env["out"][h, :, qs:qs + QC], in_=osb[:])


def _build_nc(reps=1, fin_dve=1, pump_burst=0, skip_av=0, skip_attn=0, skip_proj=0, pump_wide=0, skip_vtr=0, xt_only=0, skip_evict=0, xt_pre=0, av3=0, fin4=0, xtq=1, pace4=0, wrap=0, filler=0):
    import contextlib
    from collections import deque

    import concourse.bacc as bacc
    import concourse.mybir as mybir
    import concourse.tile as tile

    dt = mybir.dt
    AF = mybir.ActivationFunctionType
    ALU = mybir.AluOpType

    nc = bacc.Bacc(None, target_bir_lowering=False, debug=False)

    x16 = nc.declare_dram_parameter("x16c", [NCH, HIDDEN, TCH], dt.bfloat16, isOutput=False)
    wq = nc.declare_dram_parameter("wqt", [HIDDEN, P], dt.bfloat16, isOutput=False)
    wk = nc.declare_dram_parameter("wkt", [HIDDEN, P], dt.bfloat16, isOutput=False)
    wv = nc.declare_dram_parameter("wvt", [HIDDEN, P], dt.bfloat16, isOutput=False)
    bq = nc.declare_dram_parameter("bq", [P, 1], dt.float32, isOutput=False)
    bk = nc.declare_dram_parameter("bk", [P, 1], dt.float32, isOutput=False)
    bv = nc.declare_dram_parameter("bv", [P, 1], dt.float32, isOutput=False)
    out = nc.declare_dram_parameter("out", [HPC, D_K, T], dt.float32, isOutput=True)

    with tile.TileContext(nc) as tc:
        with (
            tc.tile_pool(name="const", bufs=1) as const,
            tc.tile_pool(name="persist", bufs=1) as persist,
            tc.tile_pool(name="xtp", bufs=8 if xt_pre else 4) as xtp,
            tc.tile_pool(name="vtp", bufs=2) as vtp,
            tc.tile_pool(name="ptp", bufs=3 if xt_pre else 4) as ptp,
            tc.tile_pool(name="fin", bufs=4 if fin4 else 2) as fin,
            # PSUM budget (8 banks): scores 2x2 + proj 2x1 + attended 2x1.
            tc.tile_pool(name="scp", bufs=2, space="PSUM") as scp,
            tc.tile_pool(name="pjp", bufs=1 if (pump_wide or av3) else 2, space="PSUM") as pjp,
            tc.tile_pool(name="avp", bufs=3 if av3 else 2, space="PSUM") as avp,
        ):
            # --- weights: host ships W^T [HIDDEN, 128]; plain strided DMA
            # into [hid-part, kt, 128] (no XBAR transposes needed) ---
            wts, bts = [], []
            for nm, wparam, bparam in (("q", wq, bq), ("k", wk, bk), ("v", wv, bv)):
                wt = const.tile([P, KT, P], dt.bfloat16, tag=f"wt{nm}")
                nc.sync.dma_start(
                    out=wt[:],
                    in_=wparam[:].rearrange("(k p) d -> p k d", p=P),
                )
                bt = const.tile([P, 1], dt.float32, tag=f"b{nm}")
                nc.sync.dma_start(out=bt[:], in_=bparam[:])
                wts.append(wt)
                bts.append(bt)

            qT = persist.tile([P, T], dt.bfloat16, tag="qT")
            kT = persist.tile([P, T], dt.bfloat16, tag="kT")
            vaug = persist.tile([P, T // P, 2 * (D_K + 1)], dt.bfloat16, tag="vaug")
            nc.vector.memset(vaug[:, :, D_K:D_K + 1], 1.0)
            nc.vector.memset(vaug[:, :, 2 * D_K + 1:2 * D_K + 2], 1.0)
            if skip_vtr:
                nc.vector.memset(vaug[:, :, 0:D_K], 0.5)
                nc.vector.memset(vaug[:, :, D_K + 1:2 * D_K + 1], 0.5)
            if wrap:
                nc.vector.memset(qT[:], 0.0)
                nc.vector.memset(kT[:], 0.0)
                nc.vector.memset(vaug[:, :, 0:D_K], 0.0)
                nc.vector.memset(vaug[:, :, D_K + 1:2 * D_K + 1], 0.0)
            if skip_proj or xt_only or skip_evict:
                nc.vector.memset(qT[:], 0.25)
                nc.vector.memset(kT[:], 0.25)
                nc.vector.memset(vaug[:, :, 0:D_K], 0.5)
                nc.vector.memset(vaug[:, :, D_K + 1:2 * D_K + 1], 0.5)

            env = {
                "nc": nc, "dt": dt, "AF": AF, "ALU": ALU,
                "x16": x16, "out": out,
                "wts": wts, "bts": bts,
                "qT": qT, "kT": kT, "vaug": vaug,
                "xtp": xtp, "vtp": vtp, "ptp": ptp, "fin": fin,
                "scp": scp, "pjp": pjp, "avp": avp,
                "xt": {}, "units": deque(),
                "fin_dve": fin_dve, "skip_av": skip_av, "pump_wide": pump_wide,
                "skip_vtr": skip_vtr, "skip_evict": skip_evict, "xtq": xtq,
                "pace4": pace4, "wrap": wrap, "filler": filler,
            }

            def queue_units(ch_pair):
                # K first (all keys needed at next batch's first scores),
                # V second (vaug needed by first AV), Q last (consumed
                # progressively per qchunk).
                for idx in (1, 2, 0):
                    for ch in ch_pair:
                        if pump_wide:
                            env["units"].append(_ProjUnit(env, ch, idx, None))
                        else:
                            for h2 in range(TCH // QC):
                                env["units"].append(_ProjUnit(env, ch, idx, h2))

            rep_ctx = tc.For_i(0, reps, 1) if reps > 1 else contextlib.nullcontext()
            with rep_ctx:
                # prologue: first two chunks projected back-to-back (ScalarE
                # has nothing to do yet), chunks 2-3 DMA'd in the background.
                if not skip_proj:
                    if wrap:
                        # steady-state wrap: only ch2/ch3 load here (b0 pumps
                        # them); ch0/ch1 were projected by the previous rep's
                        # b3 and reload via b2's loads each rep.
                        _load_xt(env, 2)
                        _load_xt(env, 3)
                    elif xt_pre:
                        for ch in range(NCH):
                            _load_xt(env, ch)
                    else:
                        for ch in range(4):
                            _load_xt(env, ch)
                    if not xt_only and not wrap:
                        queue_units((0, 1))
                        _pump(env, 12 * KT)

                for b in range(B):
                    # wrap mode (timing reps only): batch 3 pumps the NEXT
                    # repetition's chunk 0/1 projections, so steady-state has
                    # no projection-only prologue and no pump-free batch.
                    nxt = (2 * b + 2) % NCH if wrap else 2 * b + 2
                    if not skip_proj and not xt_only and (wrap or nxt < NCH):
                        queue_units((nxt, nxt + 1))
                    if pump_burst or skip_attn:
                        _pump(env, 12 * KT)
                    for qc in range(LQC):
                        if not skip_proj and not xt_pre:
                            if wrap:
                                if b < B - 1 and qc == 1:
                                    _load_xt(env, (nxt + 2) % NCH)
                                if b < B - 1 and qc == 2:
                                    _load_xt(env, (nxt + 3) % NCH)
                            else:
                                if qc == 1 and nxt + 2 < NCH:
                                    _load_xt(env, nxt + 2)
                                if qc == 2 and nxt + 3 < NCH:
                                    _load_xt(env, nxt + 3)
                        if not skip_attn:
                            _attn_qchunk(env, b, qc, pump_burst or skip_proj)

    nc.compile()
    return nc


def get_nc(reps=1, **kw):
    key = f"nc{reps}-{sorted(kw.items())}"
    if key not in _CACHE:
        _CACHE[key] = _build_nc(reps, **kw)
    return _CACHE[key]


def _shard_inputs(x, Wq, bq, Wk, bk, Wv, bv):
    import ml_dtypes

    x16c = np.ascontiguousarray(
        np.asarray(x, dtype=np.float32)
        .reshape(NCH, TCH, HIDDEN)
        .astype(ml_dtypes.bfloat16)
        .transpose(0, 2, 1)
    )
    in_maps = []
    for c in range(N_CORES):
        sl = slice(c * OPC, (c + 1) * OPC)
        in_maps.append({
            "x16c": x16c,
            "wqt": np.ascontiguousarray(np.asarray(Wq, dtype=np.float32)[sl].astype(ml_dtypes.bfloat16).T),
            "wkt": np.ascontiguousarray(np.asarray(Wk, dtype=np.float32)[sl].astype(ml_dtypes.bfloat16).T),
            "wvt": np.ascontiguousarray(np.asarray(Wv, dtype=np.float32)[sl].astype(ml_dtypes.bfloat16).T),
            "bq": np.ascontiguousarray(np.asarray(bq, dtype=np.float32)[sl].reshape(P, 1)),
            "bk": np.ascontiguousarray(np.asarray(bk, dtype=np.float32)[sl].reshape(P, 1)),
            "bv": np.ascontiguousarray(np.asarray(bv, dtype=np.float32)[sl].reshape(P, 1)),
        })
    return in_maps


def _gather(results):
    att = np.empty((B, NUM_HEADS, L, D_K), dtype=np.float32)
    for c in range(N_CORES):
        r = results[c]["out"]  # (HPC, D_K, T)
        for h in range(HPC):
            att[:, c * HPC + h] = r[h].T.reshape(B, L, D_K)
    return att


def run(x, Wq, bq, Wk, bk, Wv, bv, trace=False):
    from concourse.bass_utils import run_bass_kernel_spmd

    nc = get_nc()
    in_maps = _shard_inputs(x, Wq, bq, Wk, bk, Wv, bv)
    res = run_bass_kernel_spmd(
        nc, in_maps, core_ids=list(range(N_CORES)), trace=trace
    )
    return _gather(res.results), res


def kernel(x, Wq, bq, Wk, bk, Wv, bv):
    att, _ = run(x, Wq, bq, Wk, bk, Wv, bv, trace=False)
    return att
